# revision 4
# baseline (speedup 1.0000x reference)
"""Trainium2 Bass kernel for nn_PointTransformerLayer_59674275611307.

Mathematical simplification: in the reference, the attention logits `w` are
broadcast identically across the NSAMPLE axis before the softmax.  Softmax
over identical values is exactly uniform (1/16 each), and the weights sum to
exactly 1, so the grouped weighted sum of values collapses to the values
themselves:

    out = (xv_g * attn).sum(axis=1) == xv == x @ Wv + bv

(verified: rel err ~2e-7 vs the full reference).  Everything else — the q/k
projections, the position MLP, both BN+MLP stacks and the softmax — cancels
out of the output entirely.  The kernel therefore computes the single
(50000,64)@(64,64) matmul + bias, data-parallel over points across 8 cores.

Device strategy (per core, 6400 rows after padding 50000 -> 51200):
  - host packs the core's x-shard transposed into [128, 3200]: partition
    64*(t&1)+k, column (t>>1)*128 + p holds x[p*50 + t, k]  (t = 128-row
    chunk index, p = row-within-chunk).  This makes the contraction dim k
    the SBUF partition dim (required: the PE contracts over partitions)
    with zero on-device transposes, full 128-partition DMAs, and fully
    contiguous per-partition DMA descriptors.
  - 50 fp32 matmuls (lhsT = x-chunk.T [64,128] stationary, rhs = Wv [64,64]
    moving), row-packed in even/odd pairs at tile_position (0,0)/(64,0) so
    consecutive chunks run concurrently in disjoint PE row-groups writing
    separate PSUM banks.
  - DVE adds the bias while copying PSUM -> SBUF, then contiguous stores.
"""

import numpy as np

N = 50000
C = 64
NCORES = 8
T = 50                        # 128-row chunks per core
ROWS_PER_CORE = 128 * T       # 6400
N_PAD = NCORES * ROWS_PER_CORE  # 51200
PAIRS = T // 2                # 25
XT_COLS = PAIRS * 128         # 3200
GROUP = 8                     # chunks per psum group (split 4 even + 4 odd)
N_XT_PIECES = 5
PAIRS_PER_PIECE = PAIRS // N_XT_PIECES  # 5
PIECE_COLS = PAIRS_PER_PIECE * 128      # 640

# Output store pieces (columns of the [128, 3200] out-sbuf layout)
OUT_PIECES = [(0, 1024), (1024, 2048), (2048, 3200)]

TRACE = False          # test.py sets True to collect an NTFF profile
LAST_RESULT = None     # BassKernelResults of the last run (for test.py)

_cache = {}


def _get_compiled():
    if "nc" in _cache:
        return _cache["nc"]

    import concourse.mybir as mybir
    import concourse.tile as tile
    from concourse import bacc
    from concourse.bass import MemorySpace

    f32 = mybir.dt.float32
    nc = bacc.Bacc("TRN2", target_bir_lowering=False, debug=False,
                   num_devices=NCORES)

    xt_d = nc.dram_tensor("xt", [128, XT_COLS], f32, kind="ExternalInput")
    wv_d = nc.dram_tensor("wv", [128, C], f32, kind="ExternalInput")
    bias_d = nc.dram_tensor("bias", [128, GROUP * C], f32,
                            kind="ExternalInput")
    out_d = nc.dram_tensor("out", [ROWS_PER_CORE, C], f32,
                           kind="ExternalOutput")

    # out DRAM viewed as [partition p, (chunk t, channel k)]: row p*T + t
    out_pt = out_d.ap().rearrange("(p t) k -> p (t k)", p=128)

    with tile.TileContext(nc) as tc:
        with (
            tc.tile_pool(name="const", bufs=1) as constp,
            tc.tile_pool(name="xt", bufs=1) as xtp,
            tc.tile_pool(name="outp", bufs=1) as outp,
            tc.tile_pool(name="ps", bufs=3, space=MemorySpace.PSUM) as psp,
        ):
            wv = constp.tile([128, C], f32, tag="wv")
            nc.sync.dma_start(wv[:], wv_d.ap())
            bias = constp.tile([128, GROUP * C], f32, tag="bias")
            nc.sync.dma_start(bias[:], bias_d.ap())

            xt_tiles = []
            for i in range(N_XT_PIECES):
                t_ = xtp.tile([128, PIECE_COLS], f32, tag=f"xt{i}",
                              name=f"xt_sb{i}")
                nc.sync.dma_start(
                    t_[:], xt_d.ap()[:, i * PIECE_COLS:(i + 1) * PIECE_COLS])
                xt_tiles.append(t_)

            out_tiles = []
            for i, (lo, hi) in enumerate(OUT_PIECES):
                out_tiles.append(
                    outp.tile([128, hi - lo], f32, tag=f"out{i}",
                              name=f"out_sb{i}"))

            def out_piece_of(col):
                for i, (lo, hi) in enumerate(OUT_PIECES):
                    if lo <= col < hi:
                        return i, col - lo
                raise AssertionError(col)

            n_groups = (T + GROUP - 1) // GROUP
            for g in range(n_groups):
                t0 = g * GROUP
                t1 = min(t0 + GROUP, T)
                nhalf = (t1 - t0) // 2          # chunks per parity
                ps_e = psp.tile([128, 256], f32, tag="mme")
                ps_o = psp.tile([128, 256], f32, tag="mmo")
                for t in range(t0, t1):
                    a = t & 1
                    p2 = t >> 1
                    piece = p2 // PAIRS_PER_PIECE
                    local = (p2 % PAIRS_PER_PIECE) * 128
                    lhsT = xt_tiles[piece][64 * a:64 * (a + 1),
                                           local:local + 128]
                    rhs = wv[64 * a:64 * (a + 1), :]
                    j = (t - t0) >> 1
                    ps = ps_e if a == 0 else ps_o
                    nc.tensor.matmul(ps[:, j * 64:(j + 1) * 64], lhsT, rhs,
                                     start=True, stop=True)

                # bias-add PSUM -> out sbuf (even chunks then odd chunks).
                # Out cols for chunk t0+2j+a are (t0+2j+a)*64 — view the
                # group's columns at pair (128-col) granularity, then slice
                # the even/odd 64-col half of each pair.
                opi, ocol = out_piece_of(t0 * 64)
                ot = out_tiles[opi]
                width = nhalf * 64
                ot_pairs = ot[:, ocol:ocol + nhalf * 128].rearrange(
                    "p (j w) -> p j w", w=128)
                for a, ps in ((0, ps_e), (1, ps_o)):
                    dst = ot_pairs[:, :, a * 64:(a + 1) * 64]
                    src = ps[:, :width].rearrange("p (j k) -> p j k", k=64)
                    bsrc = bias[:, :width].rearrange("p (j k) -> p j k", k=64)
                    nc.vector.tensor_add(dst, src, bsrc)

            for i, (lo, hi) in enumerate(OUT_PIECES):
                nc.sync.dma_start(out_pt[:, lo:hi], out_tiles[i][:])

    nc.compile()
    _cache["nc"] = nc
    return nc


def kernel(**inputs):
    global LAST_RESULT
    x = np.asarray(inputs["x"], dtype=np.float32)
    Wv = np.asarray(inputs["Wv"], dtype=np.float32)
    bv = np.asarray(inputs["bv"], dtype=np.float32)

    nc = _get_compiled()

    x_pad = np.zeros((N_PAD, C), np.float32)
    x_pad[:N] = x
    # xt[core, 64*a + k, p2*128 + p] = x_pad[core*6400 + p*50 + (2*p2+a), k]
    xc = x_pad.reshape(NCORES, 128, PAIRS, 2, C)
    xt = np.ascontiguousarray(xc.transpose(0, 3, 4, 2, 1)).reshape(
        NCORES, 128, XT_COLS)
    wv_stack = np.ascontiguousarray(np.concatenate([Wv, Wv], axis=0))
    bias_tile = np.ascontiguousarray(
        np.broadcast_to(np.tile(bv, GROUP), (128, GROUP * C)))

    from concourse.bass_utils import run_bass_kernel_spmd
    in_maps = [{"xt": xt[i], "wv": wv_stack, "bias": bias_tile}
               for i in range(NCORES)]
    res = run_bass_kernel_spmd(nc, in_maps, list(range(NCORES)),
                               trace=TRACE)
    LAST_RESULT = res
    out = np.concatenate([res.results[i]["out"] for i in range(NCORES)],
                         axis=0)[:N]
    return np.ascontiguousarray(out)


# revision 10
# speedup vs baseline: 1.1041x; 1.1041x over previous
"""Trainium2 Bass kernel for nn_PointTransformerLayer_59674275611307.

Mathematical simplification: in the reference, the attention logits `w` are
broadcast identically across the NSAMPLE axis before the softmax.  Softmax
over identical values is exactly uniform (1/16 each), and the weights sum to
exactly 1, so the grouped weighted sum of values collapses to the values
themselves:

    out = (xv_g * attn).sum(axis=1) == xv == x @ Wv + bv

(verified: rel err ~2e-7 vs the full reference).  Everything else — the q/k
projections, the position MLP, both BN+MLP stacks and the softmax — cancels
out of the output entirely.  The kernel therefore computes the single
(50000,64)@(64,64) matmul + bias, data-parallel over points across 8 cores.

Device strategy (per core, 6400 rows after padding 50000 -> 51200):
  - host packs the core's x-shard transposed into [128, 3200]: partition
    64*(t&1)+k, column (t>>1)*128 + p holds x[p*50 + t, k]  (t = 128-row
    chunk index, p = row-within-chunk).  This makes the contraction dim k
    the SBUF partition dim (required: the PE contracts over partitions)
    with zero on-device transposes, full 128-partition DMAs, and fully
    contiguous per-partition DMA descriptors.
  - 50 fp32 matmuls (lhsT = x-chunk.T [64,128] stationary, rhs = Wv [64,64]
    moving), row-packed in even/odd pairs at tile_position (0,0)/(64,0) so
    consecutive chunks run concurrently in disjoint PE row-groups writing
    separate PSUM banks.
  - DVE adds the bias while copying PSUM -> SBUF, then contiguous stores.
"""

import numpy as np

N = 50000
C = 64
NCORES = 8
T = 50                        # 128-row chunks per core
ROWS_PER_CORE = 128 * T       # 6400
N_PAD = NCORES * ROWS_PER_CORE  # 51200
PAIRS = T // 2                # 25
XT_COLS = PAIRS * 128         # 3200
GROUP = 8                     # chunks per psum group (split 4 even + 4 odd)
N_XT_PIECES = 5
PAIRS_PER_PIECE = PAIRS // N_XT_PIECES  # 5
PIECE_COLS = PAIRS_PER_PIECE * 128      # 640

# Output store pieces (columns of the [128, 3200] out-sbuf layout).
# Final piece kept small so the tail store after the last matmul is short.
OUT_PIECES = [(0, 1024), (1024, 2048), (2048, 3072), (3072, 3200)]
N_WARMUP = 10  # dummy bf16 matmuls to engage the PE HAM clock during DMA-in

TRACE = False          # test.py sets True to collect an NTFF profile
LAST_RESULT = None     # BassKernelResults of the last run (for test.py)

_cache = {}


def _get_compiled():
    if "nc" in _cache:
        return _cache["nc"]

    import concourse.mybir as mybir
    import concourse.tile as tile
    from concourse import bacc
    from concourse.bass import MemorySpace

    f32 = mybir.dt.float32
    nc = bacc.Bacc("TRN2", target_bir_lowering=False, debug=False,
                   num_devices=NCORES)

    bf16 = mybir.dt.bfloat16
    xt_d = nc.dram_tensor("xt", [128, XT_COLS], f32, kind="ExternalInput")
    wv_d = nc.dram_tensor("wv", [128, C], f32, kind="ExternalInput")
    bias_d = nc.dram_tensor("bias", [128, C], f32, kind="ExternalInput")
    out_d = nc.dram_tensor("out", [ROWS_PER_CORE, C], f32,
                           kind="ExternalOutput")

    # out DRAM viewed as [partition p, (chunk t, channel k)]: row p*T + t
    out_pt = out_d.ap().rearrange("(p t) k -> p (t k)", p=128)

    with tile.TileContext(nc) as tc:
        with (
            tc.tile_pool(name="const", bufs=1) as constp,
            tc.tile_pool(name="xt", bufs=1) as xtp,
            tc.tile_pool(name="outp", bufs=1) as outp,
            tc.tile_pool(name="ps", bufs=3, space=MemorySpace.PSUM) as psp,
        ):
            # PE warmup: dummy bf16 matmuls with no input deps run during
            # the input-DMA window so the HAM clock gate reaches 8/8
            # before the real (fp32) matmul stream starts.
            scr = constp.tile([128, 512], bf16, tag="scr")
            nc.gpsimd.memset(scr[:], 0.0)
            ps_w = psp.tile([128, 512], f32, tag="warm", bufs=1)
            for _ in range(N_WARMUP):
                nc.tensor.matmul(ps_w[:], scr[:, :128], scr[:], start=True,
                                 stop=True)

            # Input DMAs, issue split across two HWDGE engines (sync +
            # scalar) so descriptor generation pipelines.
            wv = constp.tile([128, C], f32, tag="wv")
            nc.sync.dma_start(wv[:], wv_d.ap())
            bias = constp.tile([128, C], f32, tag="bias")
            nc.scalar.dma_start(bias[:], bias_d.ap())

            xt_tiles = []
            for i in range(N_XT_PIECES):
                t_ = xtp.tile([128, PIECE_COLS], f32, tag=f"xt{i}",
                              name=f"xt_sb{i}")
                eng = nc.sync if i % 2 == 0 else nc.scalar
                eng.dma_start(
                    t_[:], xt_d.ap()[:, i * PIECE_COLS:(i + 1) * PIECE_COLS])
                xt_tiles.append(t_)

            out_tiles = []
            for i, (lo, hi) in enumerate(OUT_PIECES):
                out_tiles.append(
                    outp.tile([128, hi - lo], f32, tag=f"out{i}",
                              name=f"out_sb{i}"))

            def out_piece_of(col):
                for i, (lo, hi) in enumerate(OUT_PIECES):
                    if lo <= col < hi:
                        return i, col - lo
                raise AssertionError(col)

            n_groups = (T + GROUP - 1) // GROUP
            for g in range(n_groups):
                t0 = g * GROUP
                t1 = min(t0 + GROUP, T)
                nhalf = (t1 - t0) // 2          # chunks per parity
                ps_e = psp.tile([128, 256], f32, tag="mme")
                ps_o = psp.tile([128, 256], f32, tag="mmo")
                for t in range(t0, t1):
                    a = t & 1
                    p2 = t >> 1
                    piece = p2 // PAIRS_PER_PIECE
                    local = (p2 % PAIRS_PER_PIECE) * 128
                    lhsT = xt_tiles[piece][64 * a:64 * (a + 1),
                                           local:local + 128]
                    rhs = wv[64 * a:64 * (a + 1), :]
                    j = (t - t0) >> 1
                    ps = ps_e if a == 0 else ps_o
                    nc.tensor.matmul(ps[:, j * 64:(j + 1) * 64], lhsT, rhs,
                                     start=True, stop=True)

                # bias-add PSUM -> out sbuf (even chunks then odd chunks).
                # Out cols for chunk t0+2j+a are (t0+2j+a)*64 — view the
                # group's columns at pair (128-col) granularity, then slice
                # the even/odd 64-col half of each pair.
                opi, ocol = out_piece_of(t0 * 64)
                ot = out_tiles[opi]
                width = nhalf * 64
                ot_pairs = ot[:, ocol:ocol + nhalf * 128].rearrange(
                    "p (j w) -> p j w", w=128)
                bsrc = bias[:, :].unsqueeze(1).broadcast_to([128, nhalf, 64])
                for a, ps in ((0, ps_e), (1, ps_o)):
                    dst = ot_pairs[:, :, a * 64:(a + 1) * 64]
                    src = ps[:, :width].rearrange("p (j k) -> p j k", k=64)
                    nc.vector.tensor_add(dst, src, bsrc)

            for i, (lo, hi) in enumerate(OUT_PIECES):
                nc.sync.dma_start(out_pt[:, lo:hi], out_tiles[i][:])

    nc.compile()
    _cache["nc"] = nc
    return nc


def kernel(**inputs):
    global LAST_RESULT
    x = np.asarray(inputs["x"], dtype=np.float32)
    Wv = np.asarray(inputs["Wv"], dtype=np.float32)
    bv = np.asarray(inputs["bv"], dtype=np.float32)

    nc = _get_compiled()

    x_pad = np.zeros((N_PAD, C), np.float32)
    x_pad[:N] = x
    # xt[core, 64*a + k, p2*128 + p] = x_pad[core*6400 + p*50 + (2*p2+a), k]
    xc = x_pad.reshape(NCORES, 128, PAIRS, 2, C)
    xt = np.ascontiguousarray(xc.transpose(0, 3, 4, 2, 1)).reshape(
        NCORES, 128, XT_COLS)
    wv_stack = np.ascontiguousarray(np.concatenate([Wv, Wv], axis=0))
    bias_tile = np.ascontiguousarray(np.broadcast_to(bv, (128, C)))

    from concourse.bass_utils import run_bass_kernel_spmd
    in_maps = [{"xt": xt[i], "wv": wv_stack, "bias": bias_tile}
               for i in range(NCORES)]
    res = run_bass_kernel_spmd(nc, in_maps, list(range(NCORES)),
                               trace=TRACE)
    LAST_RESULT = res
    out = np.concatenate([res.results[i]["out"] for i in range(NCORES)],
                         axis=0)[:N]
    return np.ascontiguousarray(out)
